# revision 19
# baseline (speedup 1.0000x reference)
"""Trainium2 Bass kernel for nn_AttentionLayer (GQA attention layer, seq=2048,
hidden=4096, 32 Q heads / 8 KV heads, head_dim=128, causal).

Sharding: one GQA group (4 Q heads + 1 K + 1 V head) per NeuronCore (8 cores).
Each core computes its group's QKV projection, causal SDPA, and a partial
output projection over its 512 output-proj contraction dims; the host sums the
8 partials.

All matmul operands are float16 (full-rate on the PE array, half the SBUF/HBM
traffic of fp32) with fp32 PSUM accumulation. Attention uses the S^T layout:
scores computed transposed [s_k, s_q] so the PV matmul needs no P-tile
transposes; no max-subtraction (scores are O(5), exp is safe). The softmax
denominator is accumulated as fp16 adds on the vector engine, partition-summed
on gpsimd, and inverted with the fast DVE reciprocal.

``_build_module(repeat=R)`` unrolls the whole computation R times inside one
NEFF (serialized by data dependencies) so device time can be measured without
per-dispatch overhead.
"""

import math

import numpy as np

SEQ = 2048
HIDDEN = 4096
HEAD_DIM = 128
N_CORES = 8
GROUP_PROJ = 768  # 4 Q heads + K + V, contiguous rows of weight_qkv per group
GROUP_E = 512  # 4 Q heads * head_dim: per-core slice of the proj contraction
SCALE = 1.0 / math.sqrt(HEAD_DIM)
OUT_F32 = True  # fp32 output partials (skips 16-bit conversion on evac path)

_RUNNER = None


def _build_module(repeat=1, phases=3):
    import concourse.bacc as bacc
    import concourse.bass_isa as bass_isa
    import concourse.mybir as mybir
    from concourse.tile import TileContext
    from concourse.masks import make_identity, make_upper_triangular

    dt = mybir.dt
    f32, f16 = dt.float32, dt.float16
    Exp = mybir.ActivationFunctionType.Exp
    Copy = mybir.ActivationFunctionType.Copy
    mult = mybir.AluOpType.mult

    nc = bacc.Bacc(None, target_bir_lowering=False)
    tok_t = nc.declare_dram_parameter("tok_t", [HIDDEN, SEQ], f16, isOutput=False)
    wq_t = nc.declare_dram_parameter("wq_t", [HIDDEN, GROUP_PROJ], f16, isOutput=False)
    wp_t = nc.declare_dram_parameter("wp_t", [GROUP_E, HIDDEN], f16, isOutput=False)
    out_dt = f32 if OUT_F32 else f16
    out_part = nc.declare_dram_parameter("out_part", [SEQ, HIDDEN], out_dt, isOutput=True)

    with TileContext(nc) as tc:
        with (
            tc.tile_pool(name="persist", bufs=1) as persist,
            tc.tile_pool(name="wq", bufs=1) as wq_pool,
            tc.tile_pool(name="p1stage", bufs=4) as stage,
            tc.tile_pool(name="attn", bufs=3) as attn_pool,
            tc.tile_pool(name="p3sb", bufs=4) as p3sb,
        ):
            # constants
            ident32 = persist.tile([128, 128], f32)
            make_identity(nc, ident32)
            ident16 = persist.tile([128, 128], f16)
            nc.vector.tensor_copy(out=ident16, in_=ident32)
            # causal keep-mask for the diagonal 128x128 block in S^T layout:
            # element (i=s_k, j=s_q) valid iff i <= j -> upper tri incl diag
            triu32 = persist.tile([128, 128], f32)
            make_upper_triangular(nc, triu32, val=1.0, diag=True)
            triu16 = persist.tile([128, 128], f16)
            nc.vector.tensor_copy(out=triu16, in_=triu32)

            # persistent activations (all fp16)
            qT = [
                [persist.tile([128, 512], f16, name=f"qT{h}_{c}") for c in range(4)]
                for h in range(4)
            ]
            kTc = [persist.tile([128, 512], f16, name=f"kTc{c}") for c in range(4)]
            v_sb = [persist.tile([128, 128], f16, name=f"v{i}") for i in range(16)]
            aoT = [
                [persist.tile([128, 512], f16, name=f"aoT{h}_{c}") for c in range(4)]
                for h in range(4)
            ]
            wp = [
                [persist.tile([128, 512], f16, name=f"wp{h}_{ck}") for ck in range(8)]
                for h in range(4)
            ]
            wq_sb = [
                wq_pool.tile([128, GROUP_PROJ], f16, name=f"wqt{kt}")
                for kt in range(32)
            ]

            for rep in range(repeat):
                # ---- phase 1: QKV projection (qkv^T layout), v transposed ----
                with (
                    tc.tile_pool(name=f"p1ps{rep}", bufs=1, space="PSUM") as p1ps,
                    tc.tile_pool(name=f"tpps{rep}", bufs=2, space="PSUM") as tp_pool,
                ):
                    for sc in range(4):
                        ps = [
                            p1ps.tile(
                                [128, 512], f32, tag=f"p1psum{pt}",
                                name=f"p1ps{pt}_{sc}",
                            )
                            for pt in range(6)
                        ]
                        for kt in range(32):
                            if sc == 0:
                                nc.sync.dma_start(
                                    out=wq_sb[kt],
                                    in_=wq_t[kt * 128 : (kt + 1) * 128, :],
                                )
                            st = stage.tile([128, 512], f16, tag="tok_stage", bufs=6)
                            nc.sync.dma_start(
                                out=st,
                                in_=tok_t[
                                    kt * 128 : (kt + 1) * 128,
                                    sc * 512 : (sc + 1) * 512,
                                ],
                            )
                            for pt in range(6):
                                nc.tensor.matmul(
                                    ps[pt],
                                    wq_sb[kt][:, pt * 128 : (pt + 1) * 128],
                                    st,
                                    start=(kt == 0),
                                    stop=(kt == 31),
                                )
                        # evacuate PSUM -> fp16 SBUF, alternating DVE/ACT
                        for h in range(4):
                            if h % 2 == 0:
                                nc.vector.tensor_copy(out=qT[h][sc], in_=ps[h])
                            else:
                                nc.scalar.activation(
                                    out=qT[h][sc], in_=ps[h], func=Copy
                                )
                        nc.scalar.activation(out=kTc[sc], in_=ps[4], func=Copy)
                        vTc = stage.tile([128, 512], f16, tag="vT_chunk")
                        nc.vector.tensor_copy(out=vTc, in_=ps[5])
                        for j in range(4):
                            ktile = sc * 4 + j
                            pst = tp_pool.tile([128, 128], f16, tag="tp")
                            nc.tensor.transpose(
                                pst, vTc[:, j * 128 : (j + 1) * 128], ident16
                            )
                            nc.vector.tensor_copy(out=v_sb[ktile], in_=pst)

                if phases < 2:
                    # keep the output written so the NEFF has a live output
                    dummy = p3sb.tile([128, 512], out_dt, tag="dummy", bufs=2)
                    nc.vector.tensor_copy(out=dummy, in_=qT[0][0])
                    nc.sync.dma_start(out=out_part[0:128, 0:512], in_=dummy)
                    continue
                # ---- phase 2: causal attention per (q-chunk, head) ----
                with (
                    tc.tile_pool(name=f"attps{rep}", bufs=1, space="PSUM") as attps,
                    tc.tile_pool(name=f"aops{rep}", bufs=1, space="PSUM") as aops,
                ):
                    # output-proj weights: DMA early, overlaps attention
                    for h in range(4):
                        for ck in range(8):
                            nc.sync.dma_start(
                                out=wp[h][ck],
                                in_=wp_t[
                                    h * 128 : (h + 1) * 128,
                                    ck * 512 : (ck + 1) * 512,
                                ],
                            )

                    def kslice(kt):
                        return kTc[kt // 4][:, (kt % 4) * 128 : (kt % 4 + 1) * 128]

                    def proj_block(st_i):
                        # out-proj for one 128-row s-tile, interleaved into
                        # the attention stream as PE filler
                        pqg, pj = st_i // 4, st_i % 4
                        for ck in range(8):
                            o_ps = aops.tile(
                                [128, 512], f32, tag="o", bufs=3,
                                name=f"o_{st_i}_{ck}",
                            )
                            for hh in range(4):
                                nc.tensor.matmul(
                                    o_ps,
                                    aoT[hh][pqg][:, pj * 128 : (pj + 1) * 128],
                                    wp[hh][ck],
                                    start=(hh == 0),
                                    stop=(hh == 3),
                                )
                            osb = p3sb.tile(
                                [128, 512], out_dt, tag="osb", bufs=10,
                                name=f"osb_{st_i}_{ck}",
                            )
                            nc.scalar.activation(out=osb, in_=o_ps, func=Copy)
                            nc.sync.dma_start(
                                out=out_part[
                                    st_i * 128 : (st_i + 1) * 128,
                                    ck * 512 : (ck + 1) * 512,
                                ],
                                in_=osb,
                            )

                    for qg in range(4):
                        for h in range(4):
                            nkt = 4 * (qg + 1)
                            nfull = 4 * qg
                            ao_ps = aops.tile([128, 512], f32, tag="ao", bufs=1)
                            ptot = attn_pool.tile([128, 512], f16, tag="ptot", bufs=2)
                            # full k-tiles: two per 2-bank PSUM tile, one exp
                            for i in range(nfull // 2):
                                kt0, kt1 = 2 * i, 2 * i + 1
                                s2 = attps.tile([128, 1024], f32, tag="s2", bufs=2)
                                nc.tensor.matmul(
                                    s2[:, 0:512], kslice(kt0), qT[h][qg],
                                    start=True, stop=True,
                                )
                                nc.tensor.matmul(
                                    s2[:, 512:1024], kslice(kt1), qT[h][qg],
                                    start=True, stop=True,
                                )
                                pT = attn_pool.tile([128, 1024], f16, tag="pT", bufs=5)
                                nc.scalar.activation(
                                    out=pT, in_=s2, func=Exp, scale=SCALE
                                )
                                nc.tensor.matmul(
                                    ao_ps, v_sb[kt0], pT[:, 0:512],
                                    start=(kt0 == 0), stop=False,
                                )
                                nc.tensor.matmul(
                                    ao_ps, v_sb[kt1], pT[:, 512:1024],
                                    start=False, stop=False,
                                )
                                if kt0 == 0:
                                    nc.vector.tensor_copy(out=ptot, in_=pT[:, 0:512])
                                else:
                                    nc.vector.tensor_add(
                                        out=ptot, in0=ptot, in1=pT[:, 0:512]
                                    )
                                nc.vector.tensor_add(
                                    out=ptot, in0=ptot, in1=pT[:, 512:1024]
                                )
                            # diagonal k-tiles: paired two-per-PSUM-tile like
                            # the full tiles; exp runs full width (columns
                            # below the causal trim hold stale-PSUM garbage
                            # whose exp output is never read)
                            for tp2 in range(2):
                                kts = (nfull + 2 * tp2, nfull + 2 * tp2 + 1)
                                s2 = attps.tile([128, 1024], f32, tag="s2", bufs=2)
                                for i, kt in enumerate(kts):
                                    c0 = (kt - nfull) * 128
                                    nc.tensor.matmul(
                                        s2[:, i * 512 + c0 : (i + 1) * 512],
                                        kslice(kt),
                                        qT[h][qg][:, c0:] if c0 else qT[h][qg],
                                        start=True,
                                        stop=True,
                                    )
                                pT = attn_pool.tile([128, 1024], f16, tag="pT", bufs=5)
                                nc.scalar.activation(
                                    out=pT, in_=s2, func=Exp, scale=SCALE
                                )
                                for i, kt in enumerate(kts):
                                    c0 = (kt - nfull) * 128
                                    lo = i * 512 + c0
                                    nc.vector.tensor_tensor(
                                        pT[:, lo : lo + 128],
                                        pT[:, lo : lo + 128],
                                        triu16,
                                        mult,
                                    )
                                    nc.tensor.matmul(
                                        ao_ps[:, c0:],
                                        v_sb[kt],
                                        pT[:, lo : (i + 1) * 512],
                                        start=(kt == 0),
                                        stop=(kt == nkt - 1),
                                    )
                                    if kt == 0:
                                        nc.vector.tensor_copy(
                                            out=ptot, in_=pT[:, 0:512]
                                        )
                                    else:
                                        nc.vector.tensor_add(
                                            out=ptot[:, c0:],
                                            in0=ptot[:, c0:],
                                            in1=pT[:, lo : (i + 1) * 512],
                                        )
                            # normalize: l = partition-sum(ptot); aoT = ao / l
                            lb = attn_pool.tile([128, 512], f32, tag="lb", bufs=2)
                            nc.gpsimd.partition_all_reduce(
                                out_ap=lb, in_ap=ptot, channels=128,
                                reduce_op=bass_isa.ReduceOp.add,
                            )
                            linv = attn_pool.tile([128, 512], f32, tag="linv", bufs=2)
                            nc.vector.reciprocal_approx_fast(out=linv, in_=lb)
                            nc.vector.tensor_tensor(aoT[h][qg], ao_ps, linv, mult)
                            if phases >= 3 and qg >= 1:
                                proj_block(4 * (qg - 1) + h)
                                if qg == 3:
                                    # qg=3's own tiles ride along here so no
                                    # projection runs unfilled after attention
                                    proj_block(12 + h)

    nc.compile()
    return nc


class _Runner:
    """Persistent jitted multi-core executor (clone of run_bass_via_pjrt)."""

    def __init__(self, nc, n_cores):
        import jax
        from jax.sharding import Mesh, PartitionSpec
        from jax.experimental.shard_map import shard_map
        import concourse.mybir as mybir
        from concourse import bass2jax

        bass2jax.install_neuronx_cc_hook()
        self.jax = jax
        self.n_cores = n_cores
        partition_name = (
            nc.partition_id_tensor.name if nc.partition_id_tensor else None
        )
        in_names, out_names, out_avals, zero_outs = [], [], [], []
        for alloc in nc.m.functions[0].allocations:
            if not isinstance(alloc, mybir.MemoryLocationSet):
                continue
            name = alloc.memorylocations[0].name
            if alloc.kind == "ExternalInput":
                if name != partition_name:
                    in_names.append(name)
            elif alloc.kind == "ExternalOutput":
                out_names.append(name)
                shape = tuple(alloc.tensor_shape)
                dtype = mybir.dt.np(alloc.dtype)
                out_avals.append(jax.core.ShapedArray(shape, dtype))
                zero_outs.append(np.zeros(shape, dtype))
        self.in_names = list(in_names)
        self.out_names = out_names
        self.out_avals = out_avals
        self.zero_outs = zero_outs
        n_params = len(in_names)
        n_outs = len(out_avals)
        all_in_names = in_names + out_names
        if partition_name is not None:
            all_in_names.append(partition_name)

        def _body(*args):
            operands = list(args)
            if partition_name is not None:
                operands.append(bass2jax.partition_id_tensor())
            outs = bass2jax._bass_exec_p.bind(
                *operands,
                out_avals=tuple(out_avals),
                in_names=tuple(all_in_names),
                out_names=tuple(out_names),
                lowering_input_output_aliases=(),
                sim_require_finite=True,
                sim_require_nnan=True,
                nc=nc,
            )
            return tuple(outs)

        self._body = _body
        self.n_params = n_params
        self.n_outs = n_outs
        devices = jax.devices()[:n_cores]
        self.mesh = Mesh(np.asarray(devices), ("core",))
        in_specs = (PartitionSpec("core"),) * (n_params + n_outs)
        out_specs = (PartitionSpec("core"),) * n_outs
        self.sharded = jax.jit(
            shard_map(
                _body,
                mesh=self.mesh,
                in_specs=in_specs,
                out_specs=out_specs,
                check_rep=False,
            ),
            donate_argnums=tuple(range(n_params, n_params + n_outs)),
            keep_unused=True,
        )

    def run(self, in_maps):
        concat_in = [
            np.concatenate(
                [np.asarray(in_maps[c][nm]) for c in range(self.n_cores)], axis=0
            )
            for nm in self.in_names
        ]
        zeros = [
            np.zeros((self.n_cores * z.shape[0], *z.shape[1:]), z.dtype)
            for z in self.zero_outs
        ]
        out_arrs = self.sharded(*concat_in, *zeros)
        return [
            {
                nm: np.asarray(out_arrs[i]).reshape(
                    self.n_cores, *self.out_avals[i].shape
                )[c]
                for i, nm in enumerate(self.out_names)
            }
            for c in range(self.n_cores)
        ]


def _get_runner():
    global _RUNNER
    if _RUNNER is None:
        nc = _build_module()
        _RUNNER = _Runner(nc, N_CORES)
    return _RUNNER


def kernel(tokens, weight_qkv, weight_proj):
    tokens = np.asarray(tokens, dtype=np.float32)
    weight_qkv = np.asarray(weight_qkv, dtype=np.float32)
    weight_proj = np.asarray(weight_proj, dtype=np.float32)

    runner = _get_runner()
    tok_t = np.ascontiguousarray(
        tokens.reshape(SEQ, HIDDEN).T.astype(np.float16)
    )
    in_maps = []
    for g in range(N_CORES):
        wq_slice = weight_qkv[g * GROUP_PROJ : (g + 1) * GROUP_PROJ, :]
        wp_slice = weight_proj[:, g * GROUP_E : (g + 1) * GROUP_E]
        in_maps.append(
            {
                "tok_t": tok_t,
                "wq_t": np.ascontiguousarray(wq_slice.T.astype(np.float16)),
                "wp_t": np.ascontiguousarray(wp_slice.T.astype(np.float16)),
            }
        )
    outs = runner.run(in_maps)
    acc = outs[0]["out_part"].astype(np.float32)
    for c in range(1, N_CORES):
        acc += outs[c]["out_part"].astype(np.float32)
    return acc.reshape(SEQ, 1, HIDDEN)
